# revision 31
# baseline (speedup 1.0000x reference)
"""Multi-headed causal attention on 8 trn2 NeuronCores (Bass/Tile).

Sharding: batch per core-PAIR, heads split within the pair.
Core c: batch c//2, heads 8*(c%2) .. 8*(c%2)+8 (4 pblocks of 2 heads),
output columns 512*(c%2) .. +512 of its batch (col-split of Wo).

All per-core differences are INPUT DATA (emb of its batch, its 8 heads'
Wq/Wk/Wv, its 512 Wo columns + bias slice); the SPMD program is
identical, so cores never need role-dependent addressing.

Per core:
  - Q^T/K^T/V^T projections per pblock (2 heads stacked on 128
    partitions), contraction over D streamed from bf16 emb chunks.
  - scores transposed ([s_k, s_q]) in bf16, per-head K=64 matmuls at
    partition offset 64*h (PE tile_position) — no zero-padding.
  - exp on ScalarE (no max subtraction; logits ~N(0,1)); causal mask
    via a precomputed tril band-multiply on VectorE; always-masked
    leading columns of diagonal tiles skipped in matmul+exp.
  - AV with V padded to 65 cols (64 V | ones): softmax denominators
    fall out as row 64 of the context accumulator; normalize via
    fast-reciprocal + GpSimd partition broadcast.
  - 2-core AllGathers (groups [[0,1],[2,3],[4,5],[6,7]]) exchange
    normalized context within the pair — only pair-local launch skew
    is absorbed, and each AG overlaps later attention. The last
    pblock's AG is split in half: the first half fires mid-attention
    (absorbing pair skew under compute) so the final AG is a short
    pure transfer, and outproj pass 2 starts almost immediately.
  - Output projection: my 512 Wo columns for ALL 2048 rows of the
    batch, contraction over the full concat dim read back from the
    AllGather outputs (slot order = natural head order, so no role
    dependence).
Proj and attention are interleaved per pblock so ScalarE exp work
overlaps the next pblock's projections; a 2-deep score->exp->AV
pipeline keeps the PE busy inside the attention loop.
"""
import sys

sys.path.insert(0, "/opt/trn_rl_repo")

import numpy as np

import concourse.bass as bass
import concourse.tile as tile
from concourse import bacc, mybir
from concourse.bass_utils import run_bass_kernel_spmd

B, S, D, H, HD = 4, 2048, 1024, 16, 64
NC_ = 8          # cores
NPB = 4          # pblocks per core (2 heads each -> 8 heads)
SC = 512         # s_q chunk (psum bank width in fp32)
NK = S // 128    # 16 s_k chunks of 128
ND = D // 128    # 8 contraction chunks of 128
F32 = mybir.dt.float32
BF16 = mybir.dt.bfloat16
FP8 = mybir.dt.float8e4
DR = mybir.MatmulPerfMode.DoubleRow
EXP = mybir.ActivationFunctionType.Exp
GE = mybir.AluOpType.is_ge
PAIRS = [[0, 1], [2, 3], [4, 5], [6, 7]]


def build():
    nc = bacc.Bacc("TRN2", target_bir_lowering=False, debug=False, num_devices=NC_)

    emb_t = nc.dram_tensor("emb_t", [ND, 128, S], BF16, kind="ExternalInput").ap()
    w_qkv = nc.dram_tensor("w_qkv", [3, NPB, ND, 128, 128], BF16,
                           kind="ExternalInput").ap()
    wo_t = nc.dram_tensor("wo_t", [ND, 128, SC], BF16, kind="ExternalInput").ap()
    bo_col = nc.dram_tensor("bo_col", [1, SC], F32, kind="ExternalInput").ap()
    out_shard = nc.dram_tensor("out_shard", [S, SC], BF16,
                               kind="ExternalOutput").ap()

    with tile.TileContext(nc) as tc:
        _build_body(nc, tc, emb_t, w_qkv, wo_t, bo_col, out_shard)

    nc.compile()
    return nc


def _build_body(nc, tc, emb_t, w_qkv, wo_t, bo_col, out_shard):
    from contextlib import ExitStack

    ctx = ExitStack()
    with ctx:
        const = ctx.enter_context(tc.tile_pool(name="const", bufs=1))
        # "mm" slots are [128, 1024] fp32 (2 PSUM banks): 3x2 + ctx 2x1 = 8
        ps_mm = ctx.enter_context(tc.tile_pool(name="ps_mm", bufs=3, space="PSUM"))
        ps_ctx = ctx.enter_context(tc.tile_pool(name="ps_ctx", bufs=2, space="PSUM"))
        dram = ctx.enter_context(tc.tile_pool(name="dram", bufs=1, space="DRAM"))

        etp = ctx.enter_context(tc.tile_pool(name="etp", bufs=1))
        qtp = ctx.enter_context(tc.tile_pool(name="qtp", bufs=1))
        ktp = ctx.enter_context(tc.tile_pool(name="ktp", bufs=1))
        vtp = ctx.enter_context(tc.tile_pool(name="vtp", bufs=2))
        vsb = ctx.enter_context(tc.tile_pool(name="vsb", bufs=2))
        # (v01 tiles: one 130-col tile per pblock, 2 generations alive)
        exp_p = ctx.enter_context(tc.tile_pool(name="exp_p", bufs=6))
        cu_p = ctx.enter_context(tc.tile_pool(name="cu_p", bufs=3))
        cn_p = ctx.enter_context(tc.tile_pool(name="cn_p", bufs=3))
        rc_p = ctx.enter_context(tc.tile_pool(name="rc_p", bufs=2))
        rb_p = ctx.enter_context(tc.tile_pool(name="rb_p", bufs=2))
        cat_p = ctx.enter_context(tc.tile_pool(name="cat_p", bufs=8))
        ob_p = ctx.enter_context(tc.tile_pool(name="ob_p", bufs=8))
        ob2_p = ctx.enter_context(tc.tile_pool(name="ob2_p", bufs=4))
        cats = {}

        # ---- input DMAs ----
        # emb per (j4, c) 512-col slices and weights per (p, pb) tiles,
        # interleaved so the first projection slot gates on ~1.3MB only.
        et = {}
        for c in range(ND):
            t = etp.tile([128, S], BF16, tag=f"et{c}", name=f"et{c}")
            et[c] = t
        w_t = {}
        for p in range(3):
            for pb in range(NPB):
                w_t[(p, pb)] = const.tile([128, ND, 128], BF16,
                                          tag=f"w{p}_{pb}", name=f"w{p}_{pb}")

        QS = (nc.sync, nc.scalar, nc.gpsimd)

        def emit_w_pb(pb, engs=(nc.gpsimd,) * 3):
            for p in range(3):
                i = NPB * p + pb
                engs[p].dma_start(
                    out=w_t[(p, pb)][:],
                    in_=bass.AP(tensor=w_qkv.tensor,
                                offset=(i * ND) * 128 * 128,
                                ap=[[128, 128], [16384, ND], [1, 128]]))

        emit_w_pb(0, QS)
        for c in range(ND):
            QS[c % 3].dma_start(out=et[c][:], in_=emb_t[c])
        for pb in range(1, NPB):
            emit_w_pb(pb)

        def wq(p, pb, c):
            return w_t[(p, pb)][:, c, :]

        wot_sb = [const.tile([128, SC], BF16, tag=f"wo{c}", name=f"wo{c}")
                  for c in range(ND)]
        for c in range(ND):
            nc.scalar.dma_start(out=wot_sb[c][:], in_=wo_t[c])

        bo_sb = const.tile([1, SC], F32, tag="bo1")
        nc.scalar.dma_start(out=bo_sb[:], in_=bo_col[:])
        bo_b2 = const.tile([128, 2 * SC], F32, tag="bob")
        nc.gpsimd.partition_broadcast(bo_b2[:, 0:SC], bo_sb[:])
        nc.gpsimd.partition_broadcast(bo_b2[:, SC:2 * SC], bo_sb[:])

        tril = const.tile([128, 128], BF16, tag="tril")
        nc.gpsimd.memset(tril[:], 1.0)
        nc.gpsimd.affine_select(out=tril[:], in_=tril[:], compare_op=GE,
                                fill=0.0, base=0, pattern=[[1, 128]],
                                channel_multiplier=-1)

        ident = const.tile([128, 128], BF16, tag="ident")
        nc.gpsimd.memset(ident[:], 1.0)
        nc.gpsimd.affine_select(out=ident[:], in_=ident[:], compare_op=GE,
                                fill=0.0, base=0, pattern=[[-1, 128]],
                                channel_multiplier=1)
        nc.gpsimd.affine_select(out=ident[:], in_=ident[:], compare_op=GE,
                                fill=0.0, base=0, pattern=[[1, 128]],
                                channel_multiplier=-1)

        ag_in = [dram.tile([128, S], BF16, tag=f"ag_in{pb}", name=f"ag_in{pb}")
                 for pb in range(NPB)]
        ag_out = [dram.tile([2, 128, S], BF16, tag=f"ag_out{pb}",
                            name=f"ag_out{pb}") for pb in range(NPB)]
        ag_in3 = [dram.tile([128, 2 * SC], BF16, tag=f"ag_in3{k}",
                            name=f"ag_in3{k}") for k in range(2)]
        ag_out3 = [dram.tile([2, 128, 2 * SC], BF16, tag=f"ag_out3{k}",
                             name=f"ag_out3{k}") for k in range(2)]

        qt = [qtp.tile([128, S], BF16, tag=f"qt{pb}", name=f"qt{pb}")
              for pb in range(NPB)]
        kt = [ktp.tile([128, S], BF16, tag=f"kt{pb}", name=f"kt{pb}")
              for pb in range(NPB)]

        # ---- per-pblock: projections + attention + pair AllGather ----
        # Projections of pblock pb+1 (and, for the last pblock, outproj
        # pass 1) are drained as generators INTO the attention m-loop so
        # the PE fills ScalarE-exp wait slots and outproj hides under
        # attention.
        v01s = {}
        P1 = [c for c in range(ND) if c % NPB != NPB - 1]
        P2 = [c for c in range(ND) if c % NPB == NPB - 1]
        obs = {}

        def emit_proj(pb):
            vt = vtp.tile([128, S], BF16, tag="vt", name=f"vt{pb}")
            for p in range(3):
                for j2 in range(2):
                    ps = ps_mm.tile([128, 2 * SC], F32, tag="mm",
                                    name=f"pj{pb}_{p}_{j2}")
                    for j4 in range(2):
                        for c in range(ND):
                            nc.tensor.matmul(
                                ps[:, SC * j4:SC * (j4 + 1)],
                                lhsT=wq(p, pb, c),
                                rhs=et[c][:, 1024 * j2 + SC * j4:
                                          1024 * j2 + SC * (j4 + 1)],
                                start=(c == 0), stop=(c == ND - 1))
                    dst = qt[pb] if p == 0 else (kt[pb] if p == 1 else vt)
                    nc.vector.tensor_copy(dst[:, 1024 * j2:1024 * (j2 + 1)],
                                          ps[:])
                    yield
            # cols: [h0 V 0:64 | one@64 | h1 V 65:129 | one@129]
            v01 = vsb.tile([128, NK, 130], BF16, tag="v01", name=f"v{pb}")
            v01s[pb] = v01
            for col in (64, 129):
                nc.vector.memset(v01[:, :, col:col + 1], 1.0)
            for sk in range(NK):
                pt = ps_mm.tile([128, 128], BF16, tag="mm",
                                name=f"tr{pb}_{sk}")
                nc.tensor.transpose(pt[:], vt[:, 128 * sk:128 * (sk + 1)],
                                    ident[:])
                for h in range(2):
                    nc.vector.tensor_copy(v01[:, sk, 65 * h:65 * h + 64],
                                          pt[:, 64 * h:64 * (h + 1)])
                if sk % 4 == 3:
                    yield

        def emit_pass1():
            for pr in range(8):         # sq pairs
                po = ps_mm.tile([128, 2 * SC], F32, tag="mm", name=f"po{pr}")
                for i, c in enumerate(P1):
                    for k in range(2):
                        sq = 2 * pr + k
                        nc.tensor.matmul(
                            po[:, SC * k:SC * (k + 1)],
                            lhsT=cats[c][:, 128 * sq:128 * (sq + 1)],
                            rhs=wot_sb[c][:],
                            start=(i == 0), stop=(i == len(P1) - 1))
                ob = ob_p.tile([128, 2 * SC], BF16, tag="ob", name=f"ob{pr}")
                nc.vector.tensor_add(ob[:], po[:], bo_b2[:])
                obs[pr] = ob
                yield

        for pb in range(NPB):
            for _ in emit_proj(pb):
                pass
            v01 = v01s[pb]
            for j in range(4):
                mtop = 4 * j + 4
                ctx_ps = [ps_ctx.tile([65, SC], F32, tag="ctx",
                                      name=f"ctx{pb}_{j}_{h}")
                          for h in range(2)]
                PIPE = 3
                exq = []

                def emit_scores(m, pb=pb, j=j, exq=exq):
                    # cols [0, c0) of each half are fully causal-masked
                    c0 = max(0, 128 * m - SC * j)
                    psc = ps_mm.tile([128, 2 * SC], F32, tag="mm",
                                     name=f"sc{pb}_{j}_{m}")
                    for h in range(2):
                        nc.tensor.matmul(
                            psc[:, SC * h + c0:SC * (h + 1)],
                            lhsT=kt[pb][64 * h:64 * (h + 1),
                                        128 * m:128 * (m + 1)],
                            rhs=qt[pb][64 * h:64 * (h + 1),
                                       SC * j + c0:SC * (j + 1)],
                            start=True, stop=True)
                    ex = exp_p.tile([128, 2 * SC], BF16, tag="ex",
                                    name=f"ex{pb}_{j}_{m}")
                    nc.scalar.activation(out=ex[:, c0:], in_=psc[:, c0:],
                                         func=EXP, scale=0.125)
                    if m >= 4 * j:  # diagonal tile: zero k>q entries
                        if c0 > 0:
                            # cols [0,c0) skipped by exp: clear stale data
                            nc.vector.memset(ex[:, 0:c0], 0.0)
                            nc.vector.memset(ex[:, SC:SC + c0], 0.0)
                        for h in range(2):
                            b0 = SC * h + c0
                            nc.vector.tensor_mul(ex[:, b0:b0 + 128],
                                                 ex[:, b0:b0 + 128], tril[:])
                    exq.append((m, ex))

                def emit_av(ctx_ps=ctx_ps, exq=exq, mtop=mtop, v01=v01):
                    m_av, ex = exq.pop(0)
                    for h in range(2):
                        nc.tensor.matmul(
                            ctx_ps[h][:],
                            lhsT=v01[:, m_av, 65 * h:65 * h + 65],
                            rhs=ex[:, SC * h:SC * (h + 1)],
                            start=(m_av == 0), stop=(m_av == mtop - 1))

                for m in range(mtop):
                    emit_scores(m)
                    if len(exq) > PIPE:
                        emit_av()
                while exq:
                    emit_av()

                # free PSUM fast, then normalize off the critical path
                for h in range(2):
                    cu = cu_p.tile([64, SC], F32, tag="cu",
                                   name=f"cu{pb}_{j}_{h}")
                    nc.vector.tensor_copy(cu[:], ctx_ps[h][0:64, :])
                    dn = rc_p.tile([1, SC], F32, tag="dn",
                                   name=f"dn{pb}_{j}_{h}")
                    nc.vector.tensor_copy(dn[:], ctx_ps[h][64:65, :])
                    rc = rc_p.tile([1, SC], F32, tag="rc")
                    nc.vector.reciprocal_approx_fast(rc[:], dn[:])
                    rb = rb_p.tile([64, SC], F32, tag="rb")
                    nc.gpsimd.partition_broadcast(rb[:], rc[:])
                    cn = cn_p.tile([64, SC], BF16, tag="cn")
                    nc.vector.tensor_mul(cn[:], cu[:], rb[:])
                    if pb < NPB - 1:
                        nc.sync.dma_start(
                            out=ag_in[pb][64 * h:64 * (h + 1),
                                          SC * j:SC * (j + 1)],
                            in_=cn[:])
                    else:
                        nc.sync.dma_start(
                            out=ag_in3[j // 2][64 * h:64 * (h + 1),
                                              SC * (j % 2):SC * (j % 2 + 1)],
                            in_=cn[:])

                if pb == NPB - 1 and j == 1:
                    # early half-AG for the last pblock: absorbs pair skew
                    # under the remaining attention
                    nc.gpsimd.collective_compute(
                        "AllGather", mybir.AluOpType.bypass,
                        replica_groups=PAIRS,
                        ins=[ag_in3[0].opt()], outs=[ag_out3[0].opt()])
                    for slot in range(2):
                        c = slot * NPB + pb
                        cats[c] = cat_p.tile([128, S], BF16, tag="cat",
                                             name=f"cat{c}")
                        nc.gpsimd.dma_start(out=cats[c][:, 0:2 * SC],
                                            in_=ag_out3[0][slot])

            if pb < NPB - 1:
                nc.gpsimd.collective_compute(
                    "AllGather", mybir.AluOpType.bypass,
                    replica_groups=PAIRS,
                    ins=[ag_in[pb].opt()], outs=[ag_out[pb].opt()])
                for slot in range(2):
                    c = slot * NPB + pb
                    ct = cat_p.tile([128, S], BF16, tag="cat", name=f"cat{c}")
                    nc.gpsimd.dma_start(out=ct[:], in_=ag_out[pb][slot])
                    cats[c] = ct
            else:
                nc.gpsimd.collective_compute(
                    "AllGather", mybir.AluOpType.bypass,
                    replica_groups=PAIRS,
                    ins=[ag_in3[1].opt()], outs=[ag_out3[1].opt()])
                # outproj pass 1 (pblock 0-2 chunks) overlaps the AG flight
                for _ in emit_pass1():
                    pass
                for slot in range(2):
                    c = slot * NPB + pb
                    eng = nc.sync if slot == 0 else nc.gpsimd
                    eng.dma_start(out=cats[c][:, 2 * SC:S],
                                  in_=ag_out3[1][slot])

        # ---- outproj pass 2: add pblock-3 chunks, bias already in ----
        for pr in (0, 1, 2, 3, 4, 5, 6, 7):
            po = ps_mm.tile([128, 2 * SC], F32, tag="mm", name=f"po2_{pr}")
            for i, c in enumerate(P2):
                for k in range(2):
                    sq = 2 * pr + k
                    nc.tensor.matmul(
                        po[:, SC * k:SC * (k + 1)],
                        lhsT=cats[c][:, 128 * sq:128 * (sq + 1)],
                        rhs=wot_sb[c][:],
                        start=(i == 0), stop=(i == len(P2) - 1))
            ob2 = ob2_p.tile([128, 2 * SC], BF16, tag="ob2", name=f"ob2_{pr}")
            nc.vector.tensor_add(ob2[:], po[:], obs[pr][:])
            for k in range(2):
                sq = 2 * pr + k
                eng = QS[sq % 3]
                eng.dma_start(
                    out=out_shard[128 * sq:128 * (sq + 1), :],
                    in_=ob2[:, SC * k:SC * (k + 1)])


_NC_CACHE = None


def _get_nc():
    global _NC_CACHE
    if _NC_CACHE is None:
        _NC_CACHE = build()
    return _NC_CACHE


def kernel(embedded, Wq, Wk, Wv, Wo, bo, _trace=False):
    import ml_dtypes

    embedded = np.asarray(embedded, np.float32)
    W = np.stack([np.asarray(Wq), np.asarray(Wk), np.asarray(Wv)]
                 ).astype(np.float32)                       # [3, H, D, HD]
    Wo = np.asarray(Wo, np.float32)
    bo = np.asarray(bo, np.float32)

    # emb per batch: [D, S] chunked [ND, 128, S]
    emb_b = [np.ascontiguousarray(embedded[p].T).astype(ml_dtypes.bfloat16)
             .reshape(ND, 128, S) for p in range(B)]
    # w per role: heads hs..hs+8 -> [3, NPB, ND, 128, 128]
    w_r = []
    for r in range(2):
        w = W[:, 8 * r:8 * r + 8]                           # [3, 8, D, HD]
        w = w.reshape(3, NPB, 2, D, HD).transpose(0, 1, 3, 2, 4)
        w = np.ascontiguousarray(w).reshape(3, NPB, ND, 128, 128)
        w_r.append(w.astype(ml_dtypes.bfloat16))
    # wo per role: my 512 output cols -> [ND, 128, SC]
    wo_r = [np.ascontiguousarray(Wo[SC * r:SC * (r + 1), :].T)
            .astype(ml_dtypes.bfloat16).reshape(ND, 128, SC) for r in range(2)]
    bo_r = [bo[SC * r:SC * (r + 1)].reshape(1, SC) for r in range(2)]

    in_maps = []
    for c in range(NC_):
        p, r = c // 2, c % 2
        in_maps.append({
            "emb_t": emb_b[p],
            "w_qkv": w_r[r],
            "wo_t": wo_r[r],
            "bo_col": bo_r[r],
        })

    nc = _get_nc()
    res = run_bass_kernel_spmd(nc, in_maps, core_ids=list(range(NC_)),
                               trace=_trace)

    out = np.empty((B, S, D), np.float32)
    for c in range(NC_):
        p, r = c // 2, c % 2
        out[p, :, SC * r:SC * (r + 1)] = np.asarray(
            res.results[c]["out_shard"]).astype(np.float32)
    if _trace:
        return out, res
    return out
